# revision 7
# baseline (speedup 1.0000x reference)
"""Trainium2 Bass kernel: Mistral quantized MLP (SwiGLU with int8-valued int32
weights, per-output-channel scales).

  gate = (x @ dequant(gate_wq).T), up = (x @ dequant(up_wq).T)
  h = silu(gate) * up
  out = h @ dequant(down_wq).T

Strategy (8 NeuronCores, tensor-parallel on the intermediate dim I):
  - Core c owns rows [c*I/8, (c+1)*I/8) of gate/up and the matching columns of
    down. Each core computes a full [H, T] partial of the down projection;
    the host sums the 8 partials (the "all-reduce"), applies down_scale, and
    transposes back to [B, S, H].

  - Precision: fp16 activations (11-bit significand) with exactly-represented
    int8 weights; one matmul pass for most of the contraction. The LAST
    N8G/N8U k-tiles (of 32) of the gate/up contractions run as single-pass
    fp8(e4m3) DoubleRow matmuls instead: both operands quantized to e4m3
    (x at scale 8, w at scale 1/8), two 128-k-tiles per instruction at 2x
    PE rate. On TRN2 hardware a DoubleRow matmul of K=256 costs the same
    cycles as an fp16 matmul of K=128, so each fp8 k-tile runs at half
    cost; with N8G=N8U=4 this trims ~4% of total PE cycles for ~1.8e-2
    end-to-end L2 error (budget 2e-2; fp16-only measures 3.6e-4).

  - Device layout keeps features on partitions, tokens on the free dim:
    x is pre-transposed/tiled on the host to [mega, ki, ko, t]; weights to
    [o_tile, ki, ko, o] so each DMA is contiguous and each matmul is
    lhsT=[128 k, 128 o] stationary x rhs=[128 k, 512 t] moving, fp32 PSUM.
    The fp8 tails are separate dram tensors in the same layout.
"""

import os

import ml_dtypes
import numpy as np

_E4 = ml_dtypes.float8_e4m3

# Problem dims (hardcoded per the task contract).
B, S, H, I = 2, 2048, 4096, 14336
NCORES = 8
I_LOC = I // NCORES  # 1792
T = B * S  # 4096
T_MEGA = 512  # tokens per resident x block (and per-matmul free dim)

# Number of trailing gate/up k-tiles (of H/128=32) computed in fp8 DoubleRow.
# Must be even (DoubleRow consumes pairs); N8U <= N8G.
N8G = int(os.environ.get("TRNMLP_N8G", "4"))
N8U = int(os.environ.get("TRNMLP_N8U", "4"))

_nc_cache = {}


def _build_module(t_mega, n_mega, ko_g, ot_g, ot_d, n8g, n8u, act_name="Silu"):
    """Build + compile the (SPMD, identical on all cores) Bass module.

    ko_g: contraction tiles for gate/up (H/128)
    ot_g: output tiles per core for gate/up (I_loc/128); also the down
          contraction tile count
    ot_d: output tiles for down (H/128)
    n8g/n8u: trailing gate/up k-tiles in fp8 (even, n8u <= n8g)
    """
    import concourse.tile as tile
    from concourse import bacc, mybir

    f32 = mybir.dt.float32
    f16 = mybir.dt.float16
    f8 = mybir.dt.float8e4
    silu = getattr(mybir.ActivationFunctionType, act_name)
    mult = mybir.AluOpType.mult
    DR = mybir.MatmulPerfMode.DoubleRow
    ko_d = ot_g
    ko16 = ko_g - n8g  # fp16 k-tiles for gate (and up: ko_g - n8u)
    assert n8g % 2 == 0 and n8u % 2 == 0 and 0 <= n8u <= n8g

    nc = bacc.Bacc(
        "TRN2",
        target_bir_lowering=False,
        debug=False,
        enable_asserts=False,
        num_devices=NCORES,
    )

    xh_d = nc.dram_tensor(
        "x_hi", [n_mega, 128, ko16 + (n8g - n8u), t_mega], f16,
        kind="ExternalInput",
    ).ap()
    x8_d = None
    if n8g:
        x8_d = nc.dram_tensor(
            "x8", [n_mega, 128, n8g, t_mega], f8, kind="ExternalInput"
        ).ap()
    gw_d = nc.dram_tensor(
        "gate_w", [ot_g, 128, ko16, 128], f16, kind="ExternalInput"
    ).ap()
    uw_d = nc.dram_tensor(
        "up_w", [ot_g, 128, ko_g - n8u, 128], f16, kind="ExternalInput"
    ).ap()
    gw8_d = uw8_d = None
    if n8g:
        gw8_d = nc.dram_tensor(
            "gate_w8", [ot_g, 128, n8g, 128], f8, kind="ExternalInput"
        ).ap()
    if n8u:
        uw8_d = nc.dram_tensor(
            "up_w8", [ot_g, 128, n8u, 128], f8, kind="ExternalInput"
        ).ap()
    dw_d = nc.dram_tensor(
        "down_w", [ot_d, 128, ko_d, 128], f16, kind="ExternalInput"
    ).ap()
    gs_d = nc.dram_tensor("gate_s", [128, ot_g], f32, kind="ExternalInput").ap()
    us_d = nc.dram_tensor("up_s", [128, ot_g], f32, kind="ExternalInput").ap()
    out_d = nc.dram_tensor(
        "out", [ot_d * 128, n_mega * t_mega], f32, kind="ExternalOutput"
    ).ap()

    with tile.TileContext(nc) as tc:
        with (
            tc.tile_pool(name="px", bufs=2) as px,
            tc.tile_pool(name="pw", bufs=3) as pw,
            tc.tile_pool(name="pdw", bufs=4) as pdw,
            tc.tile_pool(name="ph", bufs=2) as ph,
            tc.tile_pool(name="pe", bufs=2) as pe,
            tc.tile_pool(name="po", bufs=3) as po,
            tc.tile_pool(name="pscale", bufs=1) as pscale,
            tc.tile_pool(name="pp", bufs=8, space="PSUM") as pp,
        ):
            gs_t = pscale.tile([128, ot_g], f32, name="gs_t")
            nc.sync.dma_start(out=gs_t[:], in_=gs_d[:])
            us_t = pscale.tile([128, ot_g], f32, name="us_t")
            nc.sync.dma_start(out=us_t[:], in_=us_d[:])

            # x chunking: split each mega's fp16 x into NCH chunks so the
            # first matmuls start as soon as the first chunk's DMA lands
            # instead of waiting for the full 3.5MB transfer.
            ko16u = ko_g - n8u
            NCH = 4 if ko16u % 4 == 0 else 1
            CH = ko16u // NCH

            def g_group(m, ot, xcs, x8, hh):
                def xh(k):
                    return xcs[k // CH][:, k % CH, :]
                """Gate+up matmul group for (mega m, out tile ot) + SwiGLU."""
                gw = pw.tile([128, ko16, 128], f16, tag="gw", name="gw")
                nc.sync.dma_start(out=gw[:], in_=gw_d[ot])
                uw = pw.tile(
                    [128, ko_g - n8u, 128], f16, tag="uw", name="uw"
                )
                nc.sync.dma_start(out=uw[:], in_=uw_d[ot])
                if n8g:
                    gw8 = pw.tile([128, n8g, 128], f8, tag="gw8", name="gw8")
                    nc.sync.dma_start(out=gw8[:], in_=gw8_d[ot])
                if n8u:
                    uw8 = pw.tile([128, n8u, 128], f8, tag="uw8", name="uw8")
                    nc.sync.dma_start(out=uw8[:], in_=uw8_d[ot])

                psg = pp.tile([128, t_mega], f32, tag="ps", name="psg")
                for k in range(ko16):
                    nc.tensor.matmul(
                        psg[:], gw[:, k, :], xh(k),
                        start=(k == 0), stop=(n8g == 0 and k == ko16 - 1),
                    )
                for p in range(n8g // 2):
                    sl = slice(2 * p, 2 * p + 2)
                    nc.tensor.matmul(
                        psg[:], gw8[:, sl, :], x8[:, sl, :],
                        start=False, stop=(p == n8g // 2 - 1),
                        perf_mode=DR,
                    )
                psu = pp.tile([128, t_mega], f32, tag="ps", name="psu")
                for k in range(ko_g - n8u):
                    nc.tensor.matmul(
                        psu[:], uw[:, k, :], xh(k),
                        start=(k == 0), stop=(n8u == 0 and k == ko_g - n8u - 1),
                    )
                for p in range(n8u // 2):
                    # up's fp8 tiles are the LAST n8u of x8
                    sl = slice(n8g - n8u + 2 * p, n8g - n8u + 2 * p + 2)
                    nc.tensor.matmul(
                        psu[:], uw8[:, sl.start - (n8g - n8u) : sl.stop - (n8g - n8u), :],
                        x8[:, sl, :],
                        start=False, stop=(p == n8u // 2 - 1),
                        perf_mode=DR,
                    )

                gact = pe.tile([128, t_mega], f32, tag="gact", name="gact")
                nc.scalar.activation(
                    gact[:], psg[:], silu, scale=gs_t[:, ot : ot + 1]
                )
                # h = (up_psum * up_scale) * silu(gate * gate_scale)
                nc.vector.scalar_tensor_tensor(
                    hh[:, ot, :], psu[:], us_t[:, ot : ot + 1], gact[:],
                    mult, mult,
                )

            def d_group(m, o2, hh):
                """Down matmul group for (mega m, out tile o2); host scales."""
                dw = pdw.tile([128, ko_d, 128], f16, tag="dw", name="dw")
                nc.sync.dma_start(out=dw[:], in_=dw_d[o2])
                pso = pp.tile([128, t_mega], f32, tag="ps", name="pso")
                for k in range(ko_d):
                    nc.tensor.matmul(
                        pso[:], dw[:, k, :], hh[:, k, :],
                        start=(k == 0), stop=(k == ko_d - 1),
                    )
                ob = po.tile([128, t_mega], f32, tag="ob", name="ob")
                nc.scalar.copy(ob[:], pso[:])
                nc.sync.dma_start(
                    out=out_d[
                        o2 * 128 : (o2 + 1) * 128,
                        m * t_mega : (m + 1) * t_mega,
                    ],
                    in_=ob[:],
                )

            # Software pipeline: interleave mega m's gate/up groups with mega
            # m-1's down groups, spreading the down-phase DMA (down weights +
            # out stores) across the whole mega so HBM never saturates and the
            # PE never stalls.
            prev = None  # (m-1, hh)
            for m in range(n_mega):
                xcs = []
                for i in range(NCH):
                    xc = px.tile(
                        [128, CH, t_mega], f16, tag=f"xc{i}", name=f"xc{i}"
                    )
                    nc.sync.dma_start(
                        out=xc[:], in_=xh_d[m, :, i * CH : (i + 1) * CH, :]
                    )
                    xcs.append(xc)
                x8 = None
                if n8g:
                    x8 = px.tile([128, n8g, t_mega], f8, tag="x8", name="x8")
                    nc.sync.dma_start(out=x8[:], in_=x8_d[m])
                hh = ph.tile([128, ko_d, t_mega], f16, tag="hh", name="hh")

                for ot in range(ot_g):
                    g_group(m, ot, xcs, x8, hh)
                    if prev is not None:
                        pm, phh = prev
                        for o2 in range(
                            ot_d * ot // ot_g, ot_d * (ot + 1) // ot_g
                        ):
                            d_group(pm, o2, phh)
                prev = (m, hh)

            pm, phh = prev
            for o2 in range(ot_d):
                d_group(pm, o2, phh)

    nc.compile()
    return nc


def _get_module(t_mega, n_mega, ko_g, ot_g, ot_d, n8g, n8u):
    key = (t_mega, n_mega, ko_g, ot_g, ot_d, n8g, n8u)
    if key not in _nc_cache:
        _nc_cache[key] = _build_module(
            t_mega, n_mega, ko_g, ot_g, ot_d, n8g, n8u
        )
    return _nc_cache[key]


def _prep_x(x, t_mega, n_mega, ko_g, n8g, n8u):
    """[T, H] f32 -> ([mega, ki, ko16+(n8g-n8u), t] f16, [mega, ki, n8g, t] f8).

    The fp16 tensor covers k-tiles [0, ko_g - n8u); the fp8 tensor covers the
    last n8g tiles (gate uses all of them, up the last n8u; tiles in
    [ko16, ko16 + n8g - n8u) appear in BOTH, consumed as fp16 by up and as
    fp8 by gate).
    """
    ko16 = ko_g - n8g
    t_total = n_mega * t_mega
    xf = np.ascontiguousarray(x.reshape(t_total, ko_g * 128), dtype=np.float32)
    xr = xf.reshape(n_mega, t_mega, ko_g, 128).transpose(0, 3, 2, 1)
    x_hi = np.ascontiguousarray(xr[:, :, : ko_g - n8u, :]).astype(np.float16)
    x8 = None
    if n8g:
        x8 = (8.0 * np.ascontiguousarray(xr[:, :, ko16:, :])).astype(_E4)
    return x_hi, x8


def _prep_w(w_int, ot, ko, n8):
    """[ot*128 (o), ko*128 (k)] int-valued -> fp16 [ot, ki, ko-n8, o] plus
    fp8 [ot, ki, n8, o] (= w/8 on the last n8 k-tiles)."""
    w = w_int.astype(np.float32).reshape(ot, 128, ko, 128).transpose(0, 3, 2, 1)
    w16 = np.ascontiguousarray(w[:, :, : ko - n8, :]).astype(np.float16)
    if n8 == 0:
        return w16, None
    w8 = (np.ascontiguousarray(w[:, :, ko - n8 :, :]) * 0.125).astype(_E4)
    return w16, w8


def _prep_scale(s, ot):
    return np.ascontiguousarray(s.reshape(ot, 128).T, dtype=np.float32)


def _run_spmd(nc, in_maps, trace):
    from concourse.bass_utils import run_bass_kernel_spmd

    return run_bass_kernel_spmd(
        nc, in_maps, core_ids=list(range(len(in_maps))), trace=trace
    )


def kernel(x, gate_wq, gate_scale, up_wq, up_scale, down_wq, down_scale):
    n_mega = T // T_MEGA
    ko_g = H // 128
    ot_g = I_LOC // 128
    ot_d = H // 128

    nc = _get_module(T_MEGA, n_mega, ko_g, ot_g, ot_d, N8G, N8U)

    x_hi, x8 = _prep_x(np.asarray(x), T_MEGA, n_mega, ko_g, N8G, N8U)
    gate_wq = np.asarray(gate_wq)
    up_wq = np.asarray(up_wq)
    down_wq = np.asarray(down_wq)
    gate_scale = np.asarray(gate_scale, dtype=np.float32)
    up_scale = np.asarray(up_scale, dtype=np.float32)
    down_scale = np.asarray(down_scale, dtype=np.float32)

    in_maps = []
    for c in range(NCORES):
        sl = slice(c * I_LOC, (c + 1) * I_LOC)
        gw16, gw8 = _prep_w(gate_wq[sl], ot_g, ko_g, N8G)
        uw16, uw8 = _prep_w(up_wq[sl], ot_g, ko_g, N8U)
        dw16, _ = _prep_w(down_wq[:, sl], ot_d, ot_g, 0)
        im = {
            "x_hi": x_hi,
            "gate_w": gw16,
            "up_w": uw16,
            "down_w": dw16,
            "gate_s": _prep_scale(gate_scale[sl], ot_g),
            "up_s": _prep_scale(up_scale[sl], ot_g),
        }
        if x8 is not None:
            im["x8"] = x8
        if gw8 is not None:
            im["gate_w8"] = gw8
        if uw8 is not None:
            im["up_w8"] = uw8
        in_maps.append(im)

    trace = bool(int(os.environ.get("TRNMLP_TRACE", "0")))
    res = _run_spmd(nc, in_maps, trace)
    if trace:
        kernel.last_results = res

    acc = res.results[0]["out"].astype(np.float32, copy=True)
    for r in res.results[1:]:
        acc += r["out"]
    acc *= down_scale[:, None]
    return np.ascontiguousarray(acc.T).reshape(B, S, H).astype(np.float32)


kernel.last_results = None


# revision 8
# speedup vs baseline: 1.0127x; 1.0127x over previous
"""Trainium2 Bass kernel: Mistral quantized MLP (SwiGLU with int8-valued int32
weights, per-output-channel scales).

  gate = (x @ dequant(gate_wq).T), up = (x @ dequant(up_wq).T)
  h = silu(gate) * up
  out = h @ dequant(down_wq).T

Strategy (8 NeuronCores, tensor-parallel on the intermediate dim I):
  - Core c owns rows [c*I/8, (c+1)*I/8) of gate/up and the matching columns of
    down. Each core computes a full [H, T] partial of the down projection;
    the host sums the 8 partials (the "all-reduce"), applies down_scale, and
    transposes back to [B, S, H].

  - Precision: fp16 activations (11-bit significand) with exactly-represented
    int8 weights; one matmul pass for most of the contraction. The LAST
    N8G/N8U k-tiles (of 32) of the gate/up contractions run as single-pass
    fp8(e4m3) DoubleRow matmuls instead: both operands quantized to e4m3
    (x at scale 8, w at scale 1/8), two 128-k-tiles per instruction at 2x
    PE rate. On TRN2 hardware a DoubleRow matmul of K=256 costs the same
    cycles as an fp16 matmul of K=128, so each fp8 k-tile runs at half
    cost; with N8G=N8U=4 this trims ~4% of total PE cycles for ~1.8e-2
    end-to-end L2 error (budget 2e-2; fp16-only measures 3.6e-4).

  - Device layout keeps features on partitions, tokens on the free dim:
    x is pre-transposed/tiled on the host to [mega, ki, ko, t]; weights to
    [o_tile, ki, ko, o] so each DMA is contiguous and each matmul is
    lhsT=[128 k, 128 o] stationary x rhs=[128 k, 512 t] moving, fp32 PSUM.
    The fp8 tails are separate dram tensors in the same layout.
"""

import os

import ml_dtypes
import numpy as np

_E4 = ml_dtypes.float8_e4m3

# Problem dims (hardcoded per the task contract).
B, S, H, I = 2, 2048, 4096, 14336
NCORES = 8
I_LOC = I // NCORES  # 1792
T = B * S  # 4096
T_MEGA = 512  # tokens per resident x block (and per-matmul free dim)

# Number of trailing gate/up k-tiles (of H/128=32) computed in fp8 DoubleRow.
# Must be even (DoubleRow consumes pairs); N8U <= N8G.
N8G = int(os.environ.get("TRNMLP_N8G", "4"))
N8U = int(os.environ.get("TRNMLP_N8U", "4"))

_nc_cache = {}


def _build_module(t_mega, n_mega, ko_g, ot_g, ot_d, n8g, n8u, act_name="Silu"):
    """Build + compile the (SPMD, identical on all cores) Bass module.

    ko_g: contraction tiles for gate/up (H/128)
    ot_g: output tiles per core for gate/up (I_loc/128); also the down
          contraction tile count
    ot_d: output tiles for down (H/128)
    n8g/n8u: trailing gate/up k-tiles in fp8 (even, n8u <= n8g)
    """
    import concourse.tile as tile
    from concourse import bacc, mybir

    f32 = mybir.dt.float32
    f16 = mybir.dt.float16
    f8 = mybir.dt.float8e4
    silu = getattr(mybir.ActivationFunctionType, act_name)
    mult = mybir.AluOpType.mult
    DR = mybir.MatmulPerfMode.DoubleRow
    ko_d = ot_g
    ko16 = ko_g - n8g  # fp16 k-tiles for gate (and up: ko_g - n8u)
    assert n8g % 2 == 0 and n8u % 2 == 0 and 0 <= n8u <= n8g

    nc = bacc.Bacc(
        "TRN2",
        target_bir_lowering=False,
        debug=False,
        enable_asserts=False,
        num_devices=NCORES,
    )

    xh_d = nc.dram_tensor(
        "x_hi", [n_mega, 128, ko16 + (n8g - n8u), t_mega], f16,
        kind="ExternalInput",
    ).ap()
    x8_d = None
    if n8g:
        x8_d = nc.dram_tensor(
            "x8", [n_mega, 128, n8g, t_mega], f8, kind="ExternalInput"
        ).ap()
    gw_d = nc.dram_tensor(
        "gate_w", [ot_g, 128, ko16, 128], f16, kind="ExternalInput"
    ).ap()
    uw_d = nc.dram_tensor(
        "up_w", [ot_g, 128, ko_g - n8u, 128], f16, kind="ExternalInput"
    ).ap()
    gw8_d = uw8_d = None
    if n8g:
        gw8_d = nc.dram_tensor(
            "gate_w8", [ot_g, 128, n8g, 128], f8, kind="ExternalInput"
        ).ap()
    if n8u:
        uw8_d = nc.dram_tensor(
            "up_w8", [ot_g, 128, n8u, 128], f8, kind="ExternalInput"
        ).ap()
    dw_d = nc.dram_tensor(
        "down_w", [ot_d, 128, ko_d, 128], f16, kind="ExternalInput"
    ).ap()
    gs_d = nc.dram_tensor("gate_s", [128, ot_g], f32, kind="ExternalInput").ap()
    us_d = nc.dram_tensor("up_s", [128, ot_g], f32, kind="ExternalInput").ap()
    out_d = nc.dram_tensor(
        "out", [ot_d * 128, n_mega * t_mega], f32, kind="ExternalOutput"
    ).ap()

    with tile.TileContext(nc) as tc:
        with (
            tc.tile_pool(name="px", bufs=2) as px,
            tc.tile_pool(name="pw", bufs=2) as pw,
            tc.tile_pool(name="pdw", bufs=4) as pdw,
            tc.tile_pool(name="ph", bufs=2) as ph,
            tc.tile_pool(name="pe", bufs=2) as pe,
            tc.tile_pool(name="po", bufs=3) as po,
            tc.tile_pool(name="pscale", bufs=1) as pscale,
            tc.tile_pool(name="pp", bufs=8, space="PSUM") as pp,
        ):
            gs_t = pscale.tile([128, ot_g], f32, name="gs_t")
            nc.sync.dma_start(out=gs_t[:], in_=gs_d[:])
            us_t = pscale.tile([128, ot_g], f32, name="us_t")
            nc.sync.dma_start(out=us_t[:], in_=us_d[:])

            def g_group(m, ot, xh_t, x8, hh):
                """Gate+up matmul group for (mega m, out tile ot) + SwiGLU."""

                def xh(k):
                    return xh_t[:, k, :]

                gw = pw.tile([128, ko16, 128], f16, tag="gw", name="gw")
                nc.sync.dma_start(out=gw[:], in_=gw_d[ot])
                uw = pw.tile(
                    [128, ko_g - n8u, 128], f16, tag="uw", name="uw"
                )
                nc.sync.dma_start(out=uw[:], in_=uw_d[ot])
                if n8g:
                    gw8 = pw.tile([128, n8g, 128], f8, tag="gw8", name="gw8")
                    nc.sync.dma_start(out=gw8[:], in_=gw8_d[ot])
                if n8u:
                    uw8 = pw.tile([128, n8u, 128], f8, tag="uw8", name="uw8")
                    nc.sync.dma_start(out=uw8[:], in_=uw8_d[ot])

                psg = pp.tile([128, t_mega], f32, tag="ps", name="psg")
                for k in range(ko16):
                    nc.tensor.matmul(
                        psg[:], gw[:, k, :], xh(k),
                        start=(k == 0), stop=(n8g == 0 and k == ko16 - 1),
                    )
                for p in range(n8g // 2):
                    sl = slice(2 * p, 2 * p + 2)
                    nc.tensor.matmul(
                        psg[:], gw8[:, sl, :], x8[:, sl, :],
                        start=False, stop=(p == n8g // 2 - 1),
                        perf_mode=DR,
                    )
                psu = pp.tile([128, t_mega], f32, tag="ps", name="psu")
                for k in range(ko_g - n8u):
                    nc.tensor.matmul(
                        psu[:], uw[:, k, :], xh(k),
                        start=(k == 0), stop=(n8u == 0 and k == ko_g - n8u - 1),
                    )
                for p in range(n8u // 2):
                    # up's fp8 tiles are the LAST n8u of x8
                    sl = slice(n8g - n8u + 2 * p, n8g - n8u + 2 * p + 2)
                    nc.tensor.matmul(
                        psu[:], uw8[:, sl.start - (n8g - n8u) : sl.stop - (n8g - n8u), :],
                        x8[:, sl, :],
                        start=False, stop=(p == n8u // 2 - 1),
                        perf_mode=DR,
                    )

                gact = pe.tile([128, t_mega], f32, tag="gact", name="gact")
                nc.scalar.activation(
                    gact[:], psg[:], silu, scale=gs_t[:, ot : ot + 1]
                )
                # h = (up_psum * up_scale) * silu(gate * gate_scale)
                nc.vector.scalar_tensor_tensor(
                    hh[:, ot, :], psu[:], us_t[:, ot : ot + 1], gact[:],
                    mult, mult,
                )

            def d_group(m, o2, hh):
                """Down matmul group for (mega m, out tile o2); host scales."""
                dw = pdw.tile([128, ko_d, 128], f16, tag="dw", name="dw")
                nc.sync.dma_start(out=dw[:], in_=dw_d[o2])
                pso = pp.tile([128, t_mega], f32, tag="ps", name="pso")
                for k in range(ko_d):
                    nc.tensor.matmul(
                        pso[:], dw[:, k, :], hh[:, k, :],
                        start=(k == 0), stop=(k == ko_d - 1),
                    )
                ob = po.tile([128, t_mega], f32, tag="ob", name="ob")
                nc.scalar.copy(ob[:], pso[:])
                nc.sync.dma_start(
                    out=out_d[
                        o2 * 128 : (o2 + 1) * 128,
                        m * t_mega : (m + 1) * t_mega,
                    ],
                    in_=ob[:],
                )

            # Software pipeline: interleave mega m's gate/up groups with mega
            # m-1's down groups, spreading the down-phase DMA (down weights +
            # out stores) across the whole mega so HBM never saturates and the
            # PE never stalls.
            prev = None  # (m-1, hh)
            for m in range(n_mega):
                xh_t = px.tile(
                    [128, ko_g - n8u, t_mega], f16, tag="xh", name="xh"
                )
                nc.sync.dma_start(out=xh_t[:], in_=xh_d[m])
                x8 = None
                if n8g:
                    x8 = px.tile([128, n8g, t_mega], f8, tag="x8", name="x8")
                    nc.sync.dma_start(out=x8[:], in_=x8_d[m])
                hh = ph.tile([128, ko_d, t_mega], f16, tag="hh", name="hh")

                for ot in range(ot_g):
                    g_group(m, ot, xh_t, x8, hh)
                    if prev is not None:
                        pm, phh = prev
                        for o2 in range(
                            ot_d * ot // ot_g, ot_d * (ot + 1) // ot_g
                        ):
                            d_group(pm, o2, phh)
                prev = (m, hh)

            pm, phh = prev
            for o2 in range(ot_d):
                d_group(pm, o2, phh)

    nc.compile()
    return nc


def _get_module(t_mega, n_mega, ko_g, ot_g, ot_d, n8g, n8u):
    key = (t_mega, n_mega, ko_g, ot_g, ot_d, n8g, n8u)
    if key not in _nc_cache:
        _nc_cache[key] = _build_module(
            t_mega, n_mega, ko_g, ot_g, ot_d, n8g, n8u
        )
    return _nc_cache[key]


def _prep_x(x, t_mega, n_mega, ko_g, n8g, n8u):
    """[T, H] f32 -> ([mega, ki, ko16+(n8g-n8u), t] f16, [mega, ki, n8g, t] f8).

    The fp16 tensor covers k-tiles [0, ko_g - n8u); the fp8 tensor covers the
    last n8g tiles (gate uses all of them, up the last n8u; tiles in
    [ko16, ko16 + n8g - n8u) appear in BOTH, consumed as fp16 by up and as
    fp8 by gate).
    """
    ko16 = ko_g - n8g
    t_total = n_mega * t_mega
    xf = np.ascontiguousarray(x.reshape(t_total, ko_g * 128), dtype=np.float32)
    xr = xf.reshape(n_mega, t_mega, ko_g, 128).transpose(0, 3, 2, 1)
    x_hi = np.ascontiguousarray(xr[:, :, : ko_g - n8u, :]).astype(np.float16)
    x8 = None
    if n8g:
        x8 = (8.0 * np.ascontiguousarray(xr[:, :, ko16:, :])).astype(_E4)
    return x_hi, x8


def _prep_w(w_int, ot, ko, n8):
    """[ot*128 (o), ko*128 (k)] int-valued -> fp16 [ot, ki, ko-n8, o] plus
    fp8 [ot, ki, n8, o] (= w/8 on the last n8 k-tiles)."""
    w = w_int.astype(np.float32).reshape(ot, 128, ko, 128).transpose(0, 3, 2, 1)
    w16 = np.ascontiguousarray(w[:, :, : ko - n8, :]).astype(np.float16)
    if n8 == 0:
        return w16, None
    w8 = (np.ascontiguousarray(w[:, :, ko - n8 :, :]) * 0.125).astype(_E4)
    return w16, w8


def _prep_scale(s, ot):
    return np.ascontiguousarray(s.reshape(ot, 128).T, dtype=np.float32)


def _run_spmd(nc, in_maps, trace):
    from concourse.bass_utils import run_bass_kernel_spmd

    return run_bass_kernel_spmd(
        nc, in_maps, core_ids=list(range(len(in_maps))), trace=trace
    )


def kernel(x, gate_wq, gate_scale, up_wq, up_scale, down_wq, down_scale):
    n_mega = T // T_MEGA
    ko_g = H // 128
    ot_g = I_LOC // 128
    ot_d = H // 128

    nc = _get_module(T_MEGA, n_mega, ko_g, ot_g, ot_d, N8G, N8U)

    x_hi, x8 = _prep_x(np.asarray(x), T_MEGA, n_mega, ko_g, N8G, N8U)
    gate_wq = np.asarray(gate_wq)
    up_wq = np.asarray(up_wq)
    down_wq = np.asarray(down_wq)
    gate_scale = np.asarray(gate_scale, dtype=np.float32)
    up_scale = np.asarray(up_scale, dtype=np.float32)
    down_scale = np.asarray(down_scale, dtype=np.float32)

    in_maps = []
    for c in range(NCORES):
        sl = slice(c * I_LOC, (c + 1) * I_LOC)
        gw16, gw8 = _prep_w(gate_wq[sl], ot_g, ko_g, N8G)
        uw16, uw8 = _prep_w(up_wq[sl], ot_g, ko_g, N8U)
        dw16, _ = _prep_w(down_wq[:, sl], ot_d, ot_g, 0)
        im = {
            "x_hi": x_hi,
            "gate_w": gw16,
            "up_w": uw16,
            "down_w": dw16,
            "gate_s": _prep_scale(gate_scale[sl], ot_g),
            "up_s": _prep_scale(up_scale[sl], ot_g),
        }
        if x8 is not None:
            im["x8"] = x8
        if gw8 is not None:
            im["gate_w8"] = gw8
        if uw8 is not None:
            im["up_w8"] = uw8
        in_maps.append(im)

    trace = bool(int(os.environ.get("TRNMLP_TRACE", "0")))
    res = _run_spmd(nc, in_maps, trace)
    if trace:
        kernel.last_results = res

    acc = res.results[0]["out"].astype(np.float32, copy=True)
    for r in res.results[1:]:
        acc += r["out"]
    acc *= down_scale[:, None]
    return np.ascontiguousarray(acc.T).reshape(B, S, H).astype(np.float32)


kernel.last_results = None


# revision 9
# speedup vs baseline: 1.0130x; 1.0003x over previous
"""Trainium2 Bass kernel: Mistral quantized MLP (SwiGLU with int8-valued int32
weights, per-output-channel scales).

  gate = (x @ dequant(gate_wq).T), up = (x @ dequant(up_wq).T)
  h = silu(gate) * up
  out = h @ dequant(down_wq).T

Strategy (8 NeuronCores, tensor-parallel on the intermediate dim I):
  - Core c owns rows [c*I/8, (c+1)*I/8) of gate/up and the matching columns of
    down. Each core computes a full [H, T] partial of the down projection;
    the host sums the 8 partials (the "all-reduce"), applies down_scale, and
    transposes back to [B, S, H].

  - Precision: fp16 activations (11-bit significand) with exactly-represented
    int8 weights; one matmul pass for most of the contraction. The LAST
    N8G/N8U k-tiles (of 32) of the gate/up contractions run as single-pass
    fp8(e4m3) DoubleRow matmuls instead: both operands quantized to e4m3
    (x at scale 8, w at scale 1/8), two 128-k-tiles per instruction at 2x
    PE rate. On TRN2 hardware a DoubleRow matmul of K=256 costs the same
    cycles as an fp16 matmul of K=128, so each fp8 k-tile runs at half
    cost; with N8G=N8U=4 this trims ~4% of total PE cycles for ~1.8e-2
    end-to-end L2 error (budget 2e-2; fp16-only measures 3.6e-4).

  - Device layout keeps features on partitions, tokens on the free dim:
    x is pre-transposed/tiled on the host to [mega, ki, ko, t]; weights to
    [o_tile, ki, ko, o] so each DMA is contiguous and each matmul is
    lhsT=[128 k, 128 o] stationary x rhs=[128 k, 512 t] moving, fp32 PSUM.
    The fp8 tails are separate dram tensors in the same layout.
"""

import os

import ml_dtypes
import numpy as np

_E4 = ml_dtypes.float8_e4m3

# Problem dims (hardcoded per the task contract).
B, S, H, I = 2, 2048, 4096, 14336
NCORES = 8
I_LOC = I // NCORES  # 1792
T = B * S  # 4096
T_MEGA = 512  # tokens per resident x block (and per-matmul free dim)

# Number of trailing gate/up k-tiles (of H/128=32) computed in fp8 DoubleRow.
# Must be even (DoubleRow consumes pairs); N8U <= N8G.
N8G = int(os.environ.get("TRNMLP_N8G", "4"))
N8U = int(os.environ.get("TRNMLP_N8U", "4"))

_nc_cache = {}


def _build_module(t_mega, n_mega, ko_g, ot_g, ot_d, n8g, n8u, act_name="Silu"):
    """Build + compile the (SPMD, identical on all cores) Bass module.

    ko_g: contraction tiles for gate/up (H/128)
    ot_g: output tiles per core for gate/up (I_loc/128); also the down
          contraction tile count
    ot_d: output tiles for down (H/128)
    n8g/n8u: trailing gate/up k-tiles in fp8 (even, n8u <= n8g)
    """
    import concourse.tile as tile
    from concourse import bacc, mybir

    f32 = mybir.dt.float32
    f16 = mybir.dt.float16
    f8 = mybir.dt.float8e4
    silu = getattr(mybir.ActivationFunctionType, act_name)
    mult = mybir.AluOpType.mult
    DR = mybir.MatmulPerfMode.DoubleRow
    ko_d = ot_g
    ko16 = ko_g - n8g  # fp16 k-tiles for gate (and up: ko_g - n8u)
    assert n8g % 2 == 0 and n8u % 2 == 0 and 0 <= n8u <= n8g

    nc = bacc.Bacc(
        "TRN2",
        target_bir_lowering=False,
        debug=False,
        enable_asserts=False,
        num_devices=NCORES,
    )

    xh_d = nc.dram_tensor(
        "x_hi", [n_mega, 128, ko16 + (n8g - n8u), t_mega], f16,
        kind="ExternalInput",
    ).ap()
    x8_d = None
    if n8g:
        x8_d = nc.dram_tensor(
            "x8", [n_mega, 128, n8g, t_mega], f8, kind="ExternalInput"
        ).ap()
    gw_d = nc.dram_tensor(
        "gate_w", [ot_g, 128, ko16, 128], f16, kind="ExternalInput"
    ).ap()
    uw_d = nc.dram_tensor(
        "up_w", [ot_g, 128, ko_g - n8u, 128], f16, kind="ExternalInput"
    ).ap()
    gw8_d = uw8_d = None
    if n8g:
        gw8_d = nc.dram_tensor(
            "gate_w8", [ot_g, 128, n8g, 128], f8, kind="ExternalInput"
        ).ap()
    if n8u:
        uw8_d = nc.dram_tensor(
            "up_w8", [ot_g, 128, n8u, 128], f8, kind="ExternalInput"
        ).ap()
    dw_d = nc.dram_tensor(
        "down_w", [ot_d, 128, ko_d, 128], f16, kind="ExternalInput"
    ).ap()
    gs_d = nc.dram_tensor("gate_s", [128, ot_g], f32, kind="ExternalInput").ap()
    us_d = nc.dram_tensor("up_s", [128, ot_g], f32, kind="ExternalInput").ap()
    out_d = nc.dram_tensor(
        "out", [ot_d * 128, n_mega * t_mega], f32, kind="ExternalOutput"
    ).ap()

    with tile.TileContext(nc) as tc:
        with (
            tc.tile_pool(name="px", bufs=2) as px,
            tc.tile_pool(name="px0", bufs=1) as px0,
            tc.tile_pool(name="pw", bufs=2) as pw,
            tc.tile_pool(name="pdw", bufs=4) as pdw,
            tc.tile_pool(name="ph", bufs=2) as ph,
            tc.tile_pool(name="pe", bufs=2) as pe,
            tc.tile_pool(name="po", bufs=3) as po,
            tc.tile_pool(name="pscale", bufs=1) as pscale,
            tc.tile_pool(name="pp", bufs=8, space="PSUM") as pp,
        ):
            gs_t = pscale.tile([128, ot_g], f32, name="gs_t")
            nc.sync.dma_start(out=gs_t[:], in_=gs_d[:])
            us_t = pscale.tile([128, ot_g], f32, name="us_t")
            nc.sync.dma_start(out=us_t[:], in_=us_d[:])

            def g_group(m, ot, xh, x8, hh):
                """Gate+up matmul group for (mega m, out tile ot) + SwiGLU.

                xh: accessor k -> moving AP [128, t_mega] for fp16 k-tile k."""

                gw = pw.tile([128, ko16, 128], f16, tag="gw", name="gw")
                nc.sync.dma_start(out=gw[:], in_=gw_d[ot])
                uw = pw.tile(
                    [128, ko_g - n8u, 128], f16, tag="uw", name="uw"
                )
                nc.sync.dma_start(out=uw[:], in_=uw_d[ot])
                if n8g:
                    gw8 = pw.tile([128, n8g, 128], f8, tag="gw8", name="gw8")
                    nc.sync.dma_start(out=gw8[:], in_=gw8_d[ot])
                if n8u:
                    uw8 = pw.tile([128, n8u, 128], f8, tag="uw8", name="uw8")
                    nc.sync.dma_start(out=uw8[:], in_=uw8_d[ot])

                psg = pp.tile([128, t_mega], f32, tag="ps", name="psg")
                for k in range(ko16):
                    nc.tensor.matmul(
                        psg[:], gw[:, k, :], xh(k),
                        start=(k == 0), stop=(n8g == 0 and k == ko16 - 1),
                    )
                for p in range(n8g // 2):
                    sl = slice(2 * p, 2 * p + 2)
                    nc.tensor.matmul(
                        psg[:], gw8[:, sl, :], x8[:, sl, :],
                        start=False, stop=(p == n8g // 2 - 1),
                        perf_mode=DR,
                    )
                psu = pp.tile([128, t_mega], f32, tag="ps", name="psu")
                for k in range(ko_g - n8u):
                    nc.tensor.matmul(
                        psu[:], uw[:, k, :], xh(k),
                        start=(k == 0), stop=(n8u == 0 and k == ko_g - n8u - 1),
                    )
                for p in range(n8u // 2):
                    # up's fp8 tiles are the LAST n8u of x8
                    sl = slice(n8g - n8u + 2 * p, n8g - n8u + 2 * p + 2)
                    nc.tensor.matmul(
                        psu[:], uw8[:, sl.start - (n8g - n8u) : sl.stop - (n8g - n8u), :],
                        x8[:, sl, :],
                        start=False, stop=(p == n8u // 2 - 1),
                        perf_mode=DR,
                    )

                gact = pe.tile([128, t_mega], f32, tag="gact", name="gact")
                nc.scalar.activation(
                    gact[:], psg[:], silu, scale=gs_t[:, ot : ot + 1]
                )
                # h = (up_psum * up_scale) * silu(gate * gate_scale)
                nc.vector.scalar_tensor_tensor(
                    hh[:, ot, :], psu[:], us_t[:, ot : ot + 1], gact[:],
                    mult, mult,
                )

            def d_group(m, o2, hh):
                """Down matmul group for (mega m, out tile o2); host scales."""
                dw = pdw.tile([128, ko_d, 128], f16, tag="dw", name="dw")
                nc.sync.dma_start(out=dw[:], in_=dw_d[o2])
                pso = pp.tile([128, t_mega], f32, tag="ps", name="pso")
                for k in range(ko_d):
                    nc.tensor.matmul(
                        pso[:], dw[:, k, :], hh[:, k, :],
                        start=(k == 0), stop=(k == ko_d - 1),
                    )
                ob = po.tile([128, t_mega], f32, tag="ob", name="ob")
                nc.scalar.copy(ob[:], pso[:])
                nc.sync.dma_start(
                    out=out_d[
                        o2 * 128 : (o2 + 1) * 128,
                        m * t_mega : (m + 1) * t_mega,
                    ],
                    in_=ob[:],
                )

            # Software pipeline: interleave mega m's gate/up groups with mega
            # m-1's down groups, spreading the down-phase DMA (down weights +
            # out stores) across the whole mega so HBM never saturates and the
            # PE never stalls.
            # mega 0 only: split the x load into 4 parallel DMAs so the
            # first matmul starts after ~1/4 of the transfer instead of all
            # of it. Later megas keep the single-DMA steady-state pattern.
            ko16u = ko_g - n8u
            NCH = 4 if ko16u % 4 == 0 else 1
            CH = ko16u // NCH

            prev = None  # (m-1, hh)
            for m in range(n_mega):
                if m == 0 and NCH > 1:
                    xcs = []
                    for i in range(NCH):
                        xc = px0.tile(
                            [128, CH, t_mega], f16, tag=f"xc{i}", name=f"xc{i}"
                        )
                        nc.sync.dma_start(
                            out=xc[:], in_=xh_d[0, :, i * CH : (i + 1) * CH, :]
                        )
                        xcs.append(xc)

                    def xh(k, xcs=xcs):
                        return xcs[k // CH][:, k % CH, :]
                else:
                    xh_t = px.tile(
                        [128, ko16u, t_mega], f16, tag="xh", name="xh"
                    )
                    nc.sync.dma_start(out=xh_t[:], in_=xh_d[m])

                    def xh(k, xh_t=xh_t):
                        return xh_t[:, k, :]

                x8 = None
                if n8g:
                    x8 = px.tile([128, n8g, t_mega], f8, tag="x8", name="x8")
                    nc.sync.dma_start(out=x8[:], in_=x8_d[m])
                hh = ph.tile([128, ko_d, t_mega], f16, tag="hh", name="hh")

                for ot in range(ot_g):
                    g_group(m, ot, xh, x8, hh)
                    if prev is not None:
                        pm, phh = prev
                        for o2 in range(
                            ot_d * ot // ot_g, ot_d * (ot + 1) // ot_g
                        ):
                            d_group(pm, o2, phh)
                prev = (m, hh)

            pm, phh = prev
            for o2 in range(ot_d):
                d_group(pm, o2, phh)

    nc.compile()
    return nc


def _get_module(t_mega, n_mega, ko_g, ot_g, ot_d, n8g, n8u):
    key = (t_mega, n_mega, ko_g, ot_g, ot_d, n8g, n8u)
    if key not in _nc_cache:
        _nc_cache[key] = _build_module(
            t_mega, n_mega, ko_g, ot_g, ot_d, n8g, n8u
        )
    return _nc_cache[key]


def _prep_x(x, t_mega, n_mega, ko_g, n8g, n8u):
    """[T, H] f32 -> ([mega, ki, ko16+(n8g-n8u), t] f16, [mega, ki, n8g, t] f8).

    The fp16 tensor covers k-tiles [0, ko_g - n8u); the fp8 tensor covers the
    last n8g tiles (gate uses all of them, up the last n8u; tiles in
    [ko16, ko16 + n8g - n8u) appear in BOTH, consumed as fp16 by up and as
    fp8 by gate).
    """
    ko16 = ko_g - n8g
    t_total = n_mega * t_mega
    xf = np.ascontiguousarray(x.reshape(t_total, ko_g * 128), dtype=np.float32)
    xr = xf.reshape(n_mega, t_mega, ko_g, 128).transpose(0, 3, 2, 1)
    x_hi = np.ascontiguousarray(xr[:, :, : ko_g - n8u, :]).astype(np.float16)
    x8 = None
    if n8g:
        x8 = (8.0 * np.ascontiguousarray(xr[:, :, ko16:, :])).astype(_E4)
    return x_hi, x8


def _prep_w(w_int, ot, ko, n8):
    """[ot*128 (o), ko*128 (k)] int-valued -> fp16 [ot, ki, ko-n8, o] plus
    fp8 [ot, ki, n8, o] (= w/8 on the last n8 k-tiles)."""
    w = w_int.astype(np.float32).reshape(ot, 128, ko, 128).transpose(0, 3, 2, 1)
    w16 = np.ascontiguousarray(w[:, :, : ko - n8, :]).astype(np.float16)
    if n8 == 0:
        return w16, None
    w8 = (np.ascontiguousarray(w[:, :, ko - n8 :, :]) * 0.125).astype(_E4)
    return w16, w8


def _prep_scale(s, ot):
    return np.ascontiguousarray(s.reshape(ot, 128).T, dtype=np.float32)


def _run_spmd(nc, in_maps, trace):
    from concourse.bass_utils import run_bass_kernel_spmd

    return run_bass_kernel_spmd(
        nc, in_maps, core_ids=list(range(len(in_maps))), trace=trace
    )


def kernel(x, gate_wq, gate_scale, up_wq, up_scale, down_wq, down_scale):
    n_mega = T // T_MEGA
    ko_g = H // 128
    ot_g = I_LOC // 128
    ot_d = H // 128

    nc = _get_module(T_MEGA, n_mega, ko_g, ot_g, ot_d, N8G, N8U)

    x_hi, x8 = _prep_x(np.asarray(x), T_MEGA, n_mega, ko_g, N8G, N8U)
    gate_wq = np.asarray(gate_wq)
    up_wq = np.asarray(up_wq)
    down_wq = np.asarray(down_wq)
    gate_scale = np.asarray(gate_scale, dtype=np.float32)
    up_scale = np.asarray(up_scale, dtype=np.float32)
    down_scale = np.asarray(down_scale, dtype=np.float32)

    in_maps = []
    for c in range(NCORES):
        sl = slice(c * I_LOC, (c + 1) * I_LOC)
        gw16, gw8 = _prep_w(gate_wq[sl], ot_g, ko_g, N8G)
        uw16, uw8 = _prep_w(up_wq[sl], ot_g, ko_g, N8U)
        dw16, _ = _prep_w(down_wq[:, sl], ot_d, ot_g, 0)
        im = {
            "x_hi": x_hi,
            "gate_w": gw16,
            "up_w": uw16,
            "down_w": dw16,
            "gate_s": _prep_scale(gate_scale[sl], ot_g),
            "up_s": _prep_scale(up_scale[sl], ot_g),
        }
        if x8 is not None:
            im["x8"] = x8
        if gw8 is not None:
            im["gate_w8"] = gw8
        if uw8 is not None:
            im["up_w8"] = uw8
        in_maps.append(im)

    trace = bool(int(os.environ.get("TRNMLP_TRACE", "0")))
    res = _run_spmd(nc, in_maps, trace)
    if trace:
        kernel.last_results = res

    acc = res.results[0]["out"].astype(np.float32, copy=True)
    for r in res.results[1:]:
        acc += r["out"]
    acc *= down_scale[:, None]
    return np.ascontiguousarray(acc.T).reshape(B, S, H).astype(np.float32)


kernel.last_results = None
